# revision 6
# baseline (speedup 1.0000x reference)
"""Causal self-attention (B=4, T=2048, C=1024, H=16) on 8 Trainium2 NeuronCores.

Strategy: tensor-parallel over heads (2 heads per core).
Each core gets:
  - xT   [1024, 8192]  : x reshaped to [B*T, C] and transposed (c-major)
  - wT   [1024, 384]   : its 2 heads' q/k/v rows of w_attn, transposed
  - bqkv [128, 3]      : its 2 heads' q/k/v biases (column M = q/k/v chunk)
  - wpT  [128, 1024]   : its 2 heads' w_proj columns, transposed
and produces a partial output [8192, 1024]; the host sums the 8 partials
(the "all-reduce") and adds b_proj.

On-core pipeline:
  stage 1: qkvT[ch, t] = W·xT on PE (fp32r), bias added during PSUM->SBUF
           eviction on ACT.  q/k/vT stay SBUF-resident ([128, 8192] each).
  V-prep : PE-transpose of vT into V' = [V | 1] bf16 blocks ([t,d] layout,
           ones column yields softmax denominators for free).
  stage 2: per (batch, q-group of 512, head):
           ST[k, q] = KT_blk^T·QT (fp32r, causal ranges skipped, additive
           -1e9 mask on diagonal blocks), exp on ACT (scale=1/8, no
           max-subtraction: logits are ~N(0,1)) -> bf16 ET tiles,
           AV accumulation with V' into yT'[65, 512] PSUM,
           normalize via reciprocal + gpsimd partition-broadcast,
           c_proj per q-tile on PE (fp32r), DMA partial rows out.
"""
import numpy as np
from contextlib import ExitStack

import concourse.bass as bass
import concourse.bacc as bacc
import concourse.mybir as mybir
import concourse.tile as tile
from concourse import bass_utils
from concourse.masks import make_identity

FP32 = mybir.dt.float32
FP32R = mybir.dt.float32r
BF16 = mybir.dt.bfloat16
AF = mybir.ActivationFunctionType
ALU = mybir.AluOpType

B, T, C = 4, 2048, 1024
H, D = 16, 64
NCORES = 8
HPC = H // NCORES          # heads per core = 2
BT = B * T                 # 8192
NKT = T // 128             # 16 k-tiles per batch
NG = T // 512              # 4 q-groups of 512 per batch
SCALE = 1.0 / np.sqrt(D)   # 0.125
NEG = -1.0e9

_PROGRAM = None


def _build_program():
    nc = bacc.Bacc("TRN2", target_bir_lowering=False, debug=False)
    xT_d = nc.dram_tensor("xT", [C, BT], FP32R, kind="ExternalInput")
    wT_d = nc.dram_tensor("wT", [C, 3 * 128], FP32R, kind="ExternalInput")
    b_d = nc.dram_tensor("bqkv", [128, 3], FP32, kind="ExternalInput")
    wpT_d = nc.dram_tensor("wpT", [128, C], FP32R, kind="ExternalInput")
    out_d = nc.dram_tensor("out", [BT, C], FP32, kind="ExternalOutput")

    with tile.TileContext(nc) as tc, ExitStack() as ctx:
        const = ctx.enter_context(tc.tile_pool(name="const", bufs=1))
        w_sb = const.tile([128, 8, 384], FP32R)
        nc.sync.dma_start(w_sb[:], wT_d[:].rearrange("(ko p) ch -> p ko ch", p=128))
        b_sb = const.tile([128, 3], FP32)
        nc.sync.dma_start(b_sb[:], b_d[:])
        wp_sb = const.tile([128, C], FP32R)
        nc.sync.dma_start(wp_sb[:], wpT_d[:])
        ident = const.tile([128, 64], FP32)
        make_identity(nc, ident[0:64, :])
        make_identity(nc, ident[64:128, :])
        # additive causal mask for the transposed-S layout [k, q]:
        # invalid where k (partition) > q (free)
        maskT = const.tile([128, 128], FP32)
        nc.gpsimd.memset(maskT[:], NEG)
        # out[x, y] = (x - y) > 0 ? in_(NEG) : 0.0  -> NEG where k>q, else 0
        nc.gpsimd.affine_select(
            out=maskT[:], in_=maskT[:], compare_op=ALU.is_gt, fill=0.0,
            base=0, pattern=[[-1, 128]], channel_multiplier=1)

        qT_sb = const.tile([128, BT], FP32R)
        kT_sb = const.tile([128, BT], FP32R)
        vT_sb = const.tile([128, BT], FP32)
        qkv_sb = [qT_sb, kT_sb, vT_sb]

        # ---------------- stage 1: qkvT = W @ xT ----------------
        with (
            tc.tile_pool(name="xp", bufs=4) as xp,
            tc.tile_pool(name="ps1", bufs=1, space="PSUM") as ps1,
        ):
            for tch in range(BT // 512):
                psl = [ps1.tile([128, 512], FP32, name=f"p1_{M}") for M in range(3)]
                for kt in range(C // 128):
                    xt = xp.tile([128, 512], FP32R, name="xt")
                    nc.sync.dma_start(
                        xt[:], xT_d[kt * 128:(kt + 1) * 128, tch * 512:(tch + 1) * 512])
                    for M in range(3):
                        nc.tensor.matmul(
                            psl[M][:],
                            w_sb[:, kt, M * 128:(M + 1) * 128],
                            xt[:],
                            start=(kt == 0), stop=(kt == C // 128 - 1))
                for M in range(3):
                    nc.vector.tensor_scalar_add(
                        qkv_sb[M][:, tch * 512:(tch + 1) * 512], psl[M][:],
                        b_sb[:, M:M + 1])

        # ---------------- V-prep: V' = [V | 1] in [t, d] bf16 ----------------
        vp = const.tile([128, B * HPC * NKT, 65], BF16)  # slot = ((b*HPC)+h)*NKT+kt
        nc.gpsimd.memset(vp[:, :, 64:65], 1.0)
        with tc.tile_pool(name="psV", bufs=2, space="PSUM") as psV:
            for b in range(B):
                for h in range(HPC):
                    hb = h * 64
                    for kt in range(NKT):
                        pv = psV.tile([128, 64], FP32, name="pv")
                        nc.tensor.transpose(
                            pv[:],
                            vT_sb[hb:hb + 64, b * T + kt * 128:b * T + (kt + 1) * 128],
                            ident[hb:hb + 64, :])
                        nc.vector.tensor_copy(
                            vp[:, (b * HPC + h) * NKT + kt, 0:64], pv[:])

        # ---------------- stage 2: attention + c_proj ----------------
        with (
            tc.tile_pool(name="etp", bufs=5) as etp,
            tc.tile_pool(name="y2p", bufs=2) as y2p,
            tc.tile_pool(name="rp", bufs=3) as rp,
            tc.tile_pool(name="rbp", bufs=3) as rbp,
            tc.tile_pool(name="obp", bufs=3) as obp,
            tc.tile_pool(name="psS", bufs=3, space="PSUM") as psS,
            tc.tile_pool(name="psY", bufs=2, space="PSUM") as psY,
            tc.tile_pool(name="psP", bufs=2, space="PSUM") as psP,
        ):
            for b in range(B):
                t0 = b * T
                for g in range(NG):
                    q0 = t0 + g * 512
                    nkt = 4 * g + 4
                    y2 = y2p.tile([128, 512], FP32R, name="y2")
                    for h in range(HPC):
                        hb = h * 64
                        pY = psY.tile([65, 512], FP32, name="pY")
                        pend = []

                        def emit_av(kt, c0, et):
                            nc.tensor.matmul(
                                pY[:, c0:512],
                                vp[:, (b * HPC + h) * NKT + kt, :],
                                et[:, c0:512],
                                start=(kt == 0), stop=(kt == nkt - 1))

                        for kt in range(nkt):
                            c0 = max(0, kt - 4 * g) * 128
                            pS = psS.tile([128, 512], FP32, name="pS")
                            nc.tensor.matmul(
                                pS[:, c0:512],
                                kT_sb[hb:hb + 64,
                                      t0 + kt * 128:t0 + (kt + 1) * 128],
                                qT_sb[hb:hb + 64, q0 + c0:q0 + 512],
                                start=True, stop=True)
                            if kt >= 4 * g:
                                nc.vector.tensor_add(
                                    pS[:, c0:c0 + 128], pS[:, c0:c0 + 128], maskT[:])
                            et = etp.tile([128, 512], BF16, name="et")
                            nc.scalar.activation(
                                et[:, c0:512], pS[:, c0:512], AF.Exp, scale=SCALE)
                            pend.append((kt, c0, et))
                            if len(pend) > 2:
                                emit_av(*pend.pop(0))
                        for item in pend:
                            emit_av(*item)
                        # normalize: rows 0..63 of pY divided by ones-row 64
                        rinv = rp.tile([1, 512], FP32, name="rinv")
                        nc.vector.reciprocal(rinv[:], pY[64:65, :])
                        rb = rbp.tile([64, 512], FP32, name="rb")
                        nc.gpsimd.partition_broadcast(rb[:], rinv[:], channels=64)
                        nc.vector.tensor_mul(y2[hb:hb + 64, :], pY[0:64, :], rb[:])
                    # c_proj for this q-group
                    for j in range(4):
                        yls = y2[:, j * 128:(j + 1) * 128]
                        pP0 = psP.tile([128, 512], FP32, name="pP")
                        nc.tensor.matmul(pP0[:], yls, wp_sb[:, 0:512],
                                         start=True, stop=True)
                        pP1 = psP.tile([128, 512], FP32, name="pP")
                        nc.tensor.matmul(pP1[:], yls, wp_sb[:, 512:1024],
                                         start=True, stop=True)
                        ob = obp.tile([128, 1024], FP32, name="ob")
                        nc.vector.tensor_copy(ob[:, 0:512], pP0[:])
                        nc.vector.tensor_copy(ob[:, 512:1024], pP1[:])
                        row = q0 + j * 128
                        nc.sync.dma_start(out_d[row:row + 128, :], ob[:])

    nc.compile()
    return nc


def _get_program():
    global _PROGRAM
    if _PROGRAM is None:
        _PROGRAM = _build_program()
    return _PROGRAM


def _make_in_maps(x, w_attn, b_attn, w_proj):
    x = np.ascontiguousarray(np.asarray(x, dtype=np.float32))
    w_attn = np.asarray(w_attn, dtype=np.float32)
    b_attn = np.asarray(b_attn, dtype=np.float32)
    w_proj = np.asarray(w_proj, dtype=np.float32)

    xT = np.ascontiguousarray(x.reshape(BT, C).T)
    in_maps = []
    for core in range(NCORES):
        hs = [HPC * core + i for i in range(HPC)]
        wrows = np.concatenate(
            [w_attn[sec * C + h * D:sec * C + (h + 1) * D, :]
             for sec in range(3) for h in hs], axis=0)          # [384, 1024]
        wT = np.ascontiguousarray(wrows.T)                      # [1024, 384]
        brows = np.concatenate(
            [b_attn[sec * C + h * D:sec * C + (h + 1) * D]
             for sec in range(3) for h in hs], axis=0)          # [384]
        bq = np.ascontiguousarray(brows.reshape(3, 128).T)      # [128, 3]
        wpT = np.ascontiguousarray(np.concatenate(
            [w_proj[:, h * D:(h + 1) * D] for h in hs], axis=1).T)  # [128, 1024]
        in_maps.append({"xT": xT, "wT": wT, "bqkv": bq, "wpT": wpT})
    return in_maps


def run_on_hw(x, w_attn, b_attn, w_proj, b_proj, trace=False):
    """Returns (full_output [B,T,C] float32, BassKernelResults)."""
    nc = _get_program()
    in_maps = _make_in_maps(x, w_attn, b_attn, w_proj)
    res = bass_utils.run_bass_kernel_spmd(
        nc, in_maps, list(range(NCORES)), trace=trace)
    acc = np.array(res.results[0]["out"], dtype=np.float32, copy=True)
    for i in range(1, NCORES):
        acc += np.asarray(res.results[i]["out"], dtype=np.float32)
    acc += np.asarray(b_proj, dtype=np.float32)[None, :]
    return acc.reshape(B, T, C), res


def kernel(x, w_attn, b_attn, w_proj, b_proj):
    out, _ = run_on_hw(x, w_attn, b_attn, w_proj, b_proj, trace=False)
    return out
